# revision 6
# baseline (speedup 1.0000x reference)
"""Bagging autoencoder ensemble kernel for 8 Trainium2 NeuronCores.

Strategy
--------
Batch-parallel: each core gets B/8 = 512 batch rows and computes all E=100
estimators on them. Host-side weight prep removes the gather entirely
(x[:, idx[e]] @ We0[e]  ==  x @ scatter_add(We0[e], idx[e])), packs 8
estimators per matmul via concatenated / block-diagonal weights so the tiny
per-estimator layers run as dense 128-wide matmuls, and folds the final-layer
bias in via an augmented constant-one feature. All matmuls run as float32r
(FP22 multiply, fp32 accumulate) at full PE rate.

Per-core dataflow (activations kept as [feature_stack, batch] in SBUF):
  h0[128,512] = W0s_g.T @ xT          (K=256 over 2 tiles, 8 estimators)
  h1[64,512]  = blockdiag(We1).T @ h0 (+bias, relu)
  z [64,512]  = blockdiag(Wl).T @ h1  (+bias, relu)
  d0[128,512] = blockdiag(Wd0).T @ z  (+bias)
  d1[66,512]  = blockdiag-pair(Wd1aug).T @ d0 (+bias, relu; 33rd row == 1)
  o [128,512] = d1_bsub.T @ Wo_aug    (per 128-batch subtile, 2 est x 256 out)
  sigmoid -> staged [128, 2048] -> one 1 MB DMA per estimator pair
"""

import os
import sys

import numpy as np

for _p in ("/opt/trn_rl_repo", "/root/.axon_site/_ro/trn_rl_repo"):
    if os.path.isdir(_p) and _p not in sys.path:
        sys.path.append(_p)

import concourse.bass as bass
import concourse.mybir as mybir
import concourse.tile as tile
from concourse.bass_utils import run_bass_kernel_spmd

E, B, D, F, H, L = 100, 4096, 256, 32, 16, 8
N_CORES = 8
BC = B // N_CORES          # batch rows per core
G = 13                     # estimator groups of 8 (E padded 100 -> 104)
GE = 8                     # estimators per group
NPAIR = G * GE // 2        # 52 pairs incl. 2 padding pairs
NPAIR_REAL = E // 2        # 50
MA = 33                    # augmented d1 features per estimator (32 + ones row)
F32 = mybir.dt.float32
F32R = mybir.dt.float32r


def _host_prep(x, idx, We0, be0, We1, be1, Wl, bl, Wd0, bd0, Wd1, bd1, Wo, bo):
    f32 = np.float32
    x = np.ascontiguousarray(np.asarray(x, f32))
    idx = np.asarray(idx).astype(np.int64)
    We0, be0 = np.asarray(We0, f32), np.asarray(be0, f32)
    We1, be1 = np.asarray(We1, f32), np.asarray(be1, f32)
    Wl, bl = np.asarray(Wl, f32), np.asarray(bl, f32)
    Wd0, bd0 = np.asarray(Wd0, f32), np.asarray(bd0, f32)
    Wd1, bd1 = np.asarray(Wd1, f32), np.asarray(bd1, f32)
    Wo, bo = np.asarray(Wo, f32), np.asarray(bo, f32)

    # Fold the per-estimator feature gather into a scattered first-layer weight.
    W0s = np.zeros((E, D, H), f32)
    for e in range(E):
        np.add.at(W0s[e], idx[e], We0[e])

    w0s = np.zeros((128, G * 2 * 128), f32)
    b0g = np.zeros((128, G), f32)
    wb1 = np.zeros((128, G * 64), f32)
    b1g = np.zeros((64, G), f32)
    wbl = np.zeros((64, G * 64), f32)
    blg = np.zeros((64, G), f32)
    wd0 = np.zeros((64, G * 128), f32)
    bd0g = np.zeros((128, G), f32)
    for g in range(G):
        for j in range(GE):
            e = g * GE + j
            if e >= E:
                continue
            for t in range(2):
                w0s[:, (g * 2 + t) * 128 + j * H:(g * 2 + t) * 128 + (j + 1) * H] = \
                    W0s[e, t * 128:(t + 1) * 128, :]
            b0g[j * H:(j + 1) * H, g] = be0[e]
            wb1[j * H:(j + 1) * H, g * 64 + j * L:g * 64 + (j + 1) * L] = We1[e]
            b1g[j * L:(j + 1) * L, g] = be1[e]
            wbl[j * L:(j + 1) * L, g * 64 + j * L:g * 64 + (j + 1) * L] = Wl[e]
            blg[j * L:(j + 1) * L, g] = bl[e]
            wd0[j * L:(j + 1) * L, g * 128 + j * H:g * 128 + (j + 1) * H] = Wd0[e]
            bd0g[j * H:(j + 1) * H, g] = bd0[e]

    EP = G * GE  # 104 padded estimators
    wd1 = np.zeros((128, EP * MA), f32)
    bd1a = np.zeros((MA, EP), f32)
    for e in range(EP):
        j = e % GE
        if e < E:
            wd1[j * H:(j + 1) * H, e * MA:e * MA + F] = Wd1[e]
            bd1a[:F, e] = bd1[e]
        bd1a[F, e] = 1.0  # augmented constant-one feature (relu(0+1)=1)

    # dense per-pair output weight: rows = 33 aug features, cols = [e0 256 | e1 256]
    wo = np.zeros((NPAIR_REAL, MA, 2 * D), f32)
    for p in range(NPAIR_REAL):
        for c in range(2):
            e = 2 * p + c
            wo[p, :F, c * D:(c + 1) * D] = Wo[e]
            wo[p, F, c * D:(c + 1) * D] = bo[e]

    # per-core transposed x slice: [2, 128, BC], d = t*128 + r
    xts = [np.ascontiguousarray(x[c * BC:(c + 1) * BC, :].T.reshape(2, 128, BC))
           for c in range(N_CORES)]

    shared = dict(w0s=w0s, b0g=b0g, wb1=wb1, b1g=b1g, wbl=wbl, blg=blg,
                  wd0=wd0, bd0g=bd0g, wd1=wd1, bd1a=bd1a, wo=wo)
    return shared, xts


def _legalize_waits(nc, max_waits=1):
    """This neuronxcc encodes a single sem-wait slot per instruction; hoist
    overflow waits onto same-engine NoOps placed immediately before."""
    ctr = 0
    for f in nc.m.functions:
        for bb in f.blocks:
            out = []
            for inst in bb.instructions:
                si = inst.sync_info
                if si is not None and si.on_wait and len(si.on_wait) > max_waits:
                    waits = list(si.on_wait)
                    extra, keep = waits[:-max_waits], waits[-max_waits:]
                    for j in range(0, len(extra), max_waits):
                        nop = mybir.InstNoOp(name=f"I-waitsplit-{ctr}")
                        ctr += 1
                        nop.engine = inst.engine
                        nop.sync_info = mybir.SyncInfo(
                            on_wait=extra[j:j + max_waits], on_update=[])
                        out.append(nop)
                    inst.sync_info = mybir.SyncInfo(
                        on_wait=keep, on_update=list(si.on_update or []))
                out.append(inst)
            bb.instructions[:] = out


def _build_nc(legalize=True):
    nc = bass.Bass("TRN2", target_bir_lowering=False, debug=False,
                   num_devices=N_CORES)
    xt_d = nc.declare_dram_parameter("xt", [2, 128, BC], F32, isOutput=False)
    w0s_d = nc.declare_dram_parameter("w0s", [128, G * 2 * 128], F32, isOutput=False)
    b0g_d = nc.declare_dram_parameter("b0g", [128, G], F32, isOutput=False)
    wb1_d = nc.declare_dram_parameter("wb1", [128, G * 64], F32, isOutput=False)
    b1g_d = nc.declare_dram_parameter("b1g", [64, G], F32, isOutput=False)
    wbl_d = nc.declare_dram_parameter("wbl", [64, G * 64], F32, isOutput=False)
    blg_d = nc.declare_dram_parameter("blg", [64, G], F32, isOutput=False)
    wd0_d = nc.declare_dram_parameter("wd0", [64, G * 128], F32, isOutput=False)
    bd0g_d = nc.declare_dram_parameter("bd0g", [128, G], F32, isOutput=False)
    wd1_d = nc.declare_dram_parameter("wd1", [128, G * GE * MA], F32, isOutput=False)
    bd1a_d = nc.declare_dram_parameter("bd1a", [MA, G * GE], F32, isOutput=False)
    wo_d = nc.declare_dram_parameter("wo", [NPAIR_REAL, MA, 2 * D], F32, isOutput=False)
    out_d = nc.declare_dram_parameter("out", [E, BC, D], F32, isOutput=True)

    ADD = mybir.AluOpType.add
    MAX = mybir.AluOpType.max
    SIG = mybir.ActivationFunctionType.Sigmoid

    with tile.TileContext(nc) as tc:
        with (
            tc.tile_pool(name="const", bufs=1) as cp,
            tc.tile_pool(name="wop", bufs=4) as wop,
            tc.tile_pool(name="mids", bufs=2) as mids,
            tc.tile_pool(name="d1p", bufs=6) as d1p,
            tc.tile_pool(name="stage", bufs=2) as stp,
            tc.tile_pool(name="ps_mid", bufs=2, space="PSUM") as ps_mid,
            tc.tile_pool(name="ps_d1", bufs=2, space="PSUM") as ps_d1,
            tc.tile_pool(name="ps_o", bufs=4, space="PSUM") as ps_o,
        ):
            xt0 = cp.tile([128, BC], F32R, tag="xt0")
            nc.scalar.dma_start(out=xt0[:], in_=xt_d[0].bitcast(F32R))
            xt1 = cp.tile([128, BC], F32R, tag="xt1")
            nc.scalar.dma_start(out=xt1[:], in_=xt_d[1].bitcast(F32R))
            w0s_t = cp.tile([128, G * 2 * 128], F32R, tag="w0s")
            nc.scalar.dma_start(out=w0s_t[:], in_=w0s_d[:, :].bitcast(F32R))
            wb1_t = cp.tile([128, G * 64], F32R, tag="wb1")
            nc.scalar.dma_start(out=wb1_t[:], in_=wb1_d[:, :].bitcast(F32R))
            wbl_t = cp.tile([64, G * 64], F32R, tag="wbl")
            nc.scalar.dma_start(out=wbl_t[:], in_=wbl_d[:, :].bitcast(F32R))
            wd0_t = cp.tile([64, G * 128], F32R, tag="wd0")
            nc.scalar.dma_start(out=wd0_t[:], in_=wd0_d[:, :].bitcast(F32R))
            wd1_t = cp.tile([128, G * GE * MA], F32R, tag="wd1")
            nc.scalar.dma_start(out=wd1_t[:], in_=wd1_d[:, :].bitcast(F32R))
            b0_t = cp.tile([128, G], F32, tag="b0")
            nc.scalar.dma_start(out=b0_t[:], in_=b0g_d[:, :])
            b1_t = cp.tile([64, G], F32, tag="b1")
            nc.scalar.dma_start(out=b1_t[:], in_=b1g_d[:, :])
            bl_t = cp.tile([64, G], F32, tag="bl")
            nc.scalar.dma_start(out=bl_t[:], in_=blg_d[:, :])
            bd0_t = cp.tile([128, G], F32, tag="bd0")
            nc.scalar.dma_start(out=bd0_t[:], in_=bd0g_d[:, :])
            bd1_t = cp.tile([MA, G * GE], F32, tag="bd1")
            nc.scalar.dma_start(out=bd1_t[:], in_=bd1a_d[:, :])

            for g in range(G):
                ps = ps_mid.tile([128, BC], F32, tag="psm")
                nc.tensor.matmul(ps[:], w0s_t[:, (2 * g) * 128:(2 * g + 1) * 128],
                                 xt0[:], start=True, stop=False)
                nc.tensor.matmul(ps[:], w0s_t[:, (2 * g + 1) * 128:(2 * g + 2) * 128],
                                 xt1[:], start=False, stop=True)
                h0 = mids.tile([128, BC], F32R, tag="h0")
                nc.vector.tensor_scalar(h0[:], ps[:], b0_t[:, g:g + 1], None, ADD)

                ps2 = ps_mid.tile([64, BC], F32, tag="psm")
                nc.tensor.matmul(ps2[:], wb1_t[:, g * 64:(g + 1) * 64], h0[:],
                                 start=True, stop=True)
                h1 = mids.tile([64, BC], F32R, tag="h1")
                nc.vector.tensor_scalar(h1[:], ps2[:], b1_t[:, g:g + 1], 0.0, ADD, MAX)

                ps3 = ps_mid.tile([64, BC], F32, tag="psm")
                nc.tensor.matmul(ps3[:], wbl_t[:, g * 64:(g + 1) * 64], h1[:],
                                 start=True, stop=True)
                zt = mids.tile([64, BC], F32R, tag="zt")
                nc.vector.tensor_scalar(zt[:], ps3[:], bl_t[:, g:g + 1], 0.0, ADD, MAX)

                ps4 = ps_mid.tile([128, BC], F32, tag="psm")
                nc.tensor.matmul(ps4[:], wd0_t[:, g * 128:(g + 1) * 128], zt[:],
                                 start=True, stop=True)
                d0 = mids.tile([128, BC], F32R, tag="d0")
                nc.vector.tensor_scalar(d0[:], ps4[:], bd0_t[:, g:g + 1], None, ADD)

                for pl in range(4):
                    p = g * 4 + pl
                    if p >= NPAIR_REAL:
                        continue
                    wo_t = wop.tile([MA, 2 * D], F32R, tag="wo")
                    nc.scalar.dma_start(out=wo_t[:], in_=wo_d[p].bitcast(F32R))

                    d1s = []
                    for c in range(2):
                        e = 2 * p + c
                        psd = ps_d1.tile([MA, BC], F32, tag="psd")
                        nc.tensor.matmul(psd[:], wd1_t[:, e * MA:(e + 1) * MA],
                                         d0[:], start=True, stop=True)
                        d1 = d1p.tile([MA, BC], F32R, tag="d1")
                        nc.vector.tensor_scalar(d1[:], psd[:], bd1_t[:, e:e + 1],
                                                0.0, ADD, MAX)
                        d1s.append(d1)

                    stage = stp.tile([128, 2 * 4 * D], F32, tag="stage")
                    st4 = stage[:].rearrange("q (e s d) -> q e s d", e=2, s=4, d=D)
                    for s in range(4):
                        pso = ps_o.tile([128, 2 * D], F32, tag="pso")
                        for c in range(2):
                            nc.tensor.matmul(pso[:, c * D:(c + 1) * D],
                                             d1s[c][:, s * 128:(s + 1) * 128],
                                             wo_t[:, c * D:(c + 1) * D],
                                             start=True, stop=True)
                        nc.scalar.activation(st4[:, :, s, :],
                                             pso[:].rearrange("q (e d) -> q e d", e=2),
                                             SIG)
                    out_view = out_d.ap()[2 * p:2 * p + 2].rearrange(
                        "e (s q) d -> q e s d", s=4, q=128)
                    nc.sync.dma_start(out=out_view, in_=st4)

    if legalize:
        _legalize_waits(nc)
    return nc


_NC_CACHE = []


def kernel(x, idx, We0, be0, We1, be1, Wl, bl, Wd0, bd0, Wd1, bd1, Wo, bo,
           _trace=False, _trace_cores=None):
    shared, xts = _host_prep(x, idx, We0, be0, We1, be1, Wl, bl,
                             Wd0, bd0, Wd1, bd1, Wo, bo)
    if not _NC_CACHE:
        _NC_CACHE.append(_build_nc())
    nc = _NC_CACHE[0]
    in_maps = [dict(shared, xt=xts[c]) for c in range(N_CORES)]
    res = run_bass_kernel_spmd(nc, in_maps, list(range(N_CORES)),
                               trace=_trace, trace_cores=_trace_cores)
    out = np.concatenate([res.results[c]["out"] for c in range(N_CORES)], axis=1)
    if _trace:
        return out, res
    return out


# revision 7
# speedup vs baseline: 1.1871x; 1.1871x over previous
"""Bagging autoencoder ensemble kernel for 8 Trainium2 NeuronCores.

Strategy
--------
Batch-parallel: each core gets B/8 = 512 batch rows and computes all E=100
estimators on them. Host-side prep removes the gather entirely
(x[:, idx[e]] @ We0[e]  ==  x @ scatter_add(We0[e], idx[e])), folds the two
activation-free layers into their successors (W01 = W0s @ We1, Wzd1 = Wd0 @
Wd1 — exact up to fp rounding since h0/d0 have no nonlinearity), packs 8
estimators per matmul via concatenated / block-diagonal weights, and folds
the final-layer bias in via an augmented constant-one d1 feature. Matmuls
run as float32r (FP22 multiply, fp32 accumulate) at full PE rate.

Per-core dataflow (activations as [feature_stack, batch] in SBUF, batch
chunk = the core's full 512 rows):
  h1[64,512] = relu(W01_g.T @ xT + b01)      2 K-tiles, 8 estimators/matmul
  z [64,512] = relu(blockdiag(Wl).T @ h1 + bl)
  d1[66,512] = relu(blockdiag-pair(Wzd1aug).T @ z + b) (33rd row/est == 1)
  o [128,1024] = d1_bsub.T @ Wo_aug           per 128-batch subtile, pair of
                                              estimators x 256 outputs, two
                                              bsubs share a 2-bank psum
  sigmoid([128,1024]) -> stage [128,2048] -> one 1 MB DMA per pair

Engine plan: PE stream is software-pipelined (group chains emitted breadth-
first, pair d1 matmuls staggered one pair ahead of the output matmuls) so it
never stalls on DVE; input DMAs ride the idle gpsimd SWDGE queue; output
stores own the SP HWDGE ring.
"""

import os
import sys

import numpy as np

for _p in ("/opt/trn_rl_repo", "/root/.axon_site/_ro/trn_rl_repo"):
    if os.path.isdir(_p) and _p not in sys.path:
        sys.path.append(_p)

import concourse.bass as bass
import concourse.mybir as mybir
import concourse.tile as tile
from concourse.bass_utils import run_bass_kernel_spmd

E, B, D, F, H, L = 100, 4096, 256, 32, 16, 8
N_CORES = 8
BC = B // N_CORES          # batch rows per core
G = 13                     # estimator groups of 8 (E padded 100 -> 104)
GE = 8                     # estimators per group
NPAIR_REAL = E // 2        # 50 real estimator pairs
MA = 33                    # augmented d1 features per estimator (32 + ones)
F32 = mybir.dt.float32
F32R = mybir.dt.float32r


def _host_prep(x, idx, We0, be0, We1, be1, Wl, bl, Wd0, bd0, Wd1, bd1, Wo, bo):
    f32, f64 = np.float32, np.float64
    x = np.ascontiguousarray(np.asarray(x, f32))
    idx = np.asarray(idx).astype(np.int64)

    # Fold the gather into the first-layer weight, then fold the two
    # activation-free layers into their successors (in float64).
    W0s = np.zeros((E, D, H), f64)
    We0_ = np.asarray(We0, f64)
    for e in range(E):
        np.add.at(W0s[e], idx[e], We0_[e])
    W01 = np.einsum('edh,ehl->edl', W0s, np.asarray(We1, f64))          # [E,256,8]
    b01 = np.einsum('eh,ehl->el', np.asarray(be0, f64),
                    np.asarray(We1, f64)) + np.asarray(be1, f64)        # [E,8]
    Wzd1 = np.einsum('elh,ehf->elf', np.asarray(Wd0, f64),
                     np.asarray(Wd1, f64))                              # [E,8,32]
    bzd1 = np.einsum('eh,ehf->ef', np.asarray(bd0, f64),
                     np.asarray(Wd1, f64)) + np.asarray(bd1, f64)       # [E,32]
    Wl_, bl_ = np.asarray(Wl, f32), np.asarray(bl, f32)
    Wo_, bo_ = np.asarray(Wo, f32), np.asarray(bo, f32)

    w01 = np.zeros((128, G * 2 * 64), f32)    # col block (g,t): [128d, 8l x 8est]
    b01g = np.zeros((64, G), f32)
    wbl = np.zeros((64, G * 64), f32)
    blg = np.zeros((64, G), f32)
    for g in range(G):
        for j in range(GE):
            e = g * GE + j
            if e >= E:
                continue
            for t in range(2):
                w01[:, (g * 2 + t) * 64 + j * L:(g * 2 + t) * 64 + (j + 1) * L] = \
                    W01[e, t * 128:(t + 1) * 128, :]
            b01g[j * L:(j + 1) * L, g] = b01[e]
            wbl[j * L:(j + 1) * L, g * 64 + j * L:g * 64 + (j + 1) * L] = Wl_[e]
            blg[j * L:(j + 1) * L, g] = bl_[e]

    # per-pair block-diag d1 weight over the group z stack: [64, 66]
    wzd1 = np.zeros((64, NPAIR_REAL * 2 * MA), f32)
    bzd1a = np.zeros((2 * MA, NPAIR_REAL), f32)
    for p in range(NPAIR_REAL):
        g, j0 = p // 4, (p % 4) * 2
        for c in range(2):
            j = j0 + c
            e = g * GE + j
            wzd1[j * L:(j + 1) * L,
                 p * 2 * MA + c * MA:p * 2 * MA + c * MA + F] = Wzd1[e]
            bzd1a[c * MA:c * MA + F, p] = bzd1[e]
            bzd1a[c * MA + F, p] = 1.0   # relu(0 + 1) = 1 -> folds bo in

    # block-diag pair output weight [66, 512]: rows c*33..+33 -> cols c*256..+256
    wo = np.zeros((NPAIR_REAL, 2 * MA, 2 * D), f32)
    for p in range(NPAIR_REAL):
        for c in range(2):
            e = 2 * p + c
            wo[p, c * MA:c * MA + F, c * D:(c + 1) * D] = Wo_[e]
            wo[p, c * MA + F, c * D:(c + 1) * D] = bo_[e]

    xts = [np.ascontiguousarray(x[c * BC:(c + 1) * BC, :].T.reshape(2, 128, BC))
           for c in range(N_CORES)]

    shared = dict(w01=w01, b01g=b01g, wbl=wbl, blg=blg,
                  wzd1=wzd1, bzd1a=bzd1a, wo=wo)
    return shared, xts


def _legalize_waits(nc, max_waits=1):
    """This neuronxcc encodes a single sem-wait slot per instruction; hoist
    overflow waits onto same-engine NoOps placed immediately before."""
    ctr = 0
    for f in nc.m.functions:
        for bb in f.blocks:
            out = []
            for inst in bb.instructions:
                si = inst.sync_info
                if si is not None and si.on_wait and len(si.on_wait) > max_waits:
                    waits = list(si.on_wait)
                    extra, keep = waits[:-max_waits], waits[-max_waits:]
                    for j in range(0, len(extra), max_waits):
                        nop = mybir.InstNoOp(name=f"I-waitsplit-{ctr}")
                        ctr += 1
                        nop.engine = inst.engine
                        nop.sync_info = mybir.SyncInfo(
                            on_wait=extra[j:j + max_waits], on_update=[])
                        out.append(nop)
                    inst.sync_info = mybir.SyncInfo(
                        on_wait=keep, on_update=list(si.on_update or []))
                out.append(inst)
            bb.instructions[:] = out


def _build_nc(legalize=True):
    nc = bass.Bass("TRN2", target_bir_lowering=False, debug=False,
                   num_devices=N_CORES)
    xt_d = nc.declare_dram_parameter("xt", [2, 128, BC], F32, isOutput=False)
    w01_d = nc.declare_dram_parameter("w01", [128, G * 2 * 64], F32, isOutput=False)
    b01g_d = nc.declare_dram_parameter("b01g", [64, G], F32, isOutput=False)
    wbl_d = nc.declare_dram_parameter("wbl", [64, G * 64], F32, isOutput=False)
    blg_d = nc.declare_dram_parameter("blg", [64, G], F32, isOutput=False)
    wzd1_d = nc.declare_dram_parameter("wzd1", [64, NPAIR_REAL * 2 * MA], F32,
                                       isOutput=False)
    bzd1a_d = nc.declare_dram_parameter("bzd1a", [2 * MA, NPAIR_REAL], F32,
                                        isOutput=False)
    wo_d = nc.declare_dram_parameter("wo", [NPAIR_REAL, 2 * MA, 2 * D], F32,
                                     isOutput=False)
    out_d = nc.declare_dram_parameter("out", [E, BC, D], F32, isOutput=True)

    ADD = mybir.AluOpType.add
    MAX = mybir.AluOpType.max
    SIG = mybir.ActivationFunctionType.Sigmoid

    with tile.TileContext(nc) as tc:
        with (
            tc.tile_pool(name="const", bufs=1) as cp,
            tc.tile_pool(name="acts", bufs=1) as acts,
            tc.tile_pool(name="wop", bufs=6) as wop,
            tc.tile_pool(name="d1p", bufs=4) as d1p,
            tc.tile_pool(name="stage", bufs=3) as stp,
            tc.tile_pool(name="ps_mid", bufs=2, space="PSUM") as ps_mid,
            tc.tile_pool(name="ps_d1", bufs=2, space="PSUM") as ps_d1,
            tc.tile_pool(name="ps_o", bufs=2, space="PSUM") as ps_o,
        ):
            # ---- resident inputs (gpsimd SWDGE queue; SP ring is stores-only)
            xt0 = cp.tile([128, BC], F32R, tag="xt0")
            nc.gpsimd.dma_start(out=xt0[:], in_=xt_d[0].bitcast(F32R))
            xt1 = cp.tile([128, BC], F32R, tag="xt1")
            nc.gpsimd.dma_start(out=xt1[:], in_=xt_d[1].bitcast(F32R))
            w01_t = cp.tile([128, G * 2 * 64], F32R, tag="w01")
            nc.gpsimd.dma_start(out=w01_t[:], in_=w01_d[:, :].bitcast(F32R))
            wbl_t = cp.tile([64, G * 64], F32R, tag="wbl")
            nc.gpsimd.dma_start(out=wbl_t[:], in_=wbl_d[:, :].bitcast(F32R))
            wzd1_t = cp.tile([64, NPAIR_REAL * 2 * MA], F32R, tag="wzd1")
            nc.gpsimd.dma_start(out=wzd1_t[:], in_=wzd1_d[:, :].bitcast(F32R))
            b01_t = cp.tile([64, G], F32, tag="b01")
            nc.gpsimd.dma_start(out=b01_t[:], in_=b01g_d[:, :])
            bl_t = cp.tile([64, G], F32, tag="bl")
            nc.gpsimd.dma_start(out=bl_t[:], in_=blg_d[:, :])
            bzd1_t = cp.tile([2 * MA, NPAIR_REAL], F32, tag="bzd1")
            nc.gpsimd.dma_start(out=bzd1_t[:], in_=bzd1a_d[:, :])

            # ---- phase A: all first-layer matmuls breadth-first (PE gapless)
            h1s = []
            for g in range(G):
                ps = ps_mid.tile([64, BC], F32, tag="psm")
                nc.tensor.matmul(ps[:], w01_t[:, (2 * g) * 64:(2 * g + 1) * 64],
                                 xt0[:], start=True, stop=False)
                nc.tensor.matmul(ps[:], w01_t[:, (2 * g + 1) * 64:(2 * g + 2) * 64],
                                 xt1[:], start=False, stop=True)
                h1 = acts.tile([64, BC], F32R, tag=f"h1_{g}")
                nc.vector.tensor_scalar(h1[:], ps[:], b01_t[:, g:g + 1], 0.0, ADD, MAX)
                h1s.append(h1)

            # ---- phase B: all z matmuls
            zs = []
            for g in range(G):
                ps = ps_mid.tile([64, BC], F32, tag="psm")
                nc.tensor.matmul(ps[:], wbl_t[:, g * 64:(g + 1) * 64], h1s[g][:],
                                 start=True, stop=True)
                zt = acts.tile([64, BC], F32R, tag=f"z_{g}")
                nc.vector.tensor_scalar(zt[:], ps[:], bl_t[:, g:g + 1], 0.0, ADD, MAX)
                zs.append(zt)

            # ---- phase C/D: pair loop, d1 staggered one pair ahead of o
            def emit_d1(p):
                g = p // 4
                psd = ps_d1.tile([2 * MA, BC], F32, tag="psd")
                nc.tensor.matmul(psd[:], wzd1_t[:, p * 2 * MA:(p + 1) * 2 * MA],
                                 zs[g][:], start=True, stop=True)
                d1 = d1p.tile([2 * MA, BC], F32R, tag="d1")
                nc.vector.tensor_scalar(d1[:], psd[:], bzd1_t[:, p:p + 1],
                                        0.0, ADD, MAX)
                wo_t = wop.tile([2 * MA, 2 * D], F32R, tag="wo")
                nc.gpsimd.dma_start(out=wo_t[:], in_=wo_d[p].bitcast(F32R))
                return d1, wo_t

            def emit_o(p, d1, wo_t):
                stage = stp.tile([128, 2 * 4 * D], F32, tag="stage")
                st4 = stage[:].rearrange("q (e s d) -> q e s d", e=2, s=4, d=D)
                for sh in range(2):            # two bsubs per 2-bank psum
                    pso = ps_o.tile([128, 2 * 2 * D], F32, tag="pso")
                    for si in range(2):
                        s = 2 * sh + si
                        nc.tensor.matmul(pso[:, si * 2 * D:(si + 1) * 2 * D],
                                         d1[:, s * 128:(s + 1) * 128], wo_t[:],
                                         start=True, stop=True)
                    nc.scalar.activation(
                        st4[:, :, 2 * sh:2 * sh + 2, :],
                        pso[:].rearrange("q (s e d) -> q e s d", s=2, e=2, d=D),
                        SIG)
                out_view = out_d.ap()[2 * p:2 * p + 2].rearrange(
                    "e (s q) d -> q e s d", s=4, q=128)
                nc.sync.dma_start(out=out_view, in_=st4)

            pending = emit_d1(0)
            for p in range(NPAIR_REAL):
                nxt = emit_d1(p + 1) if p + 1 < NPAIR_REAL else None
                emit_o(p, *pending)
                pending = nxt

    if legalize:
        _legalize_waits(nc)
    return nc


_NC_CACHE = []


def kernel(x, idx, We0, be0, We1, be1, Wl, bl, Wd0, bd0, Wd1, bd1, Wo, bo,
           _trace=False, _trace_cores=None):
    shared, xts = _host_prep(x, idx, We0, be0, We1, be1, Wl, bl,
                             Wd0, bd0, Wd1, bd1, Wo, bo)
    if not _NC_CACHE:
        _NC_CACHE.append(_build_nc())
    nc = _NC_CACHE[0]
    in_maps = [dict(shared, xt=xts[c]) for c in range(N_CORES)]
    res = run_bass_kernel_spmd(nc, in_maps, list(range(N_CORES)),
                               trace=_trace, trace_cores=_trace_cores)
    out = np.concatenate([res.results[c]["out"] for c in range(N_CORES)], axis=1)
    if _trace:
        return out, res
    return out
